# revision 4
# baseline (speedup 1.0000x reference)
"""Trainium2 Bass kernel v14 for nn_ExpertParallelFrontBlock (MoE top-2 + front FFN).

Expert-parallel: 1 expert per core (8 cores). v5 vs v4:
- Only ident/ghl/selbig consts precede the gate stream; everything else
  (incl. the 2MB bias broadcast) lands during the W flood, so gate chunk 0
  completes ~15us earlier.
- Routing mask chain split in halves and emitted inside the gate loop:
  tokens 0:2048 are masked/counted while chunks 4-7 still stream.
- W quarters 0/1 released behind gate chunk 5, quarters 2/3 behind chunk 7.
- Fused gather/transpose/FFN(g=0) pipeline per c-tile, then FFN(g=1).
- FFN single fp32r pass, bias folded into PSUM->SBUF copies.

self-contained: hardcodes all shapes from the problem spec.
"""
import numpy as np
import ml_dtypes

import concourse.bass as bass
import concourse.mybir as mybir
import concourse.tile as tile
from concourse import bacc
from concourse.bass_utils import run_bass_kernel_spmd

F32 = mybir.dt.float32
F32R = mybir.dt.float32r
BF16 = mybir.dt.bfloat16
I32 = mybir.dt.int32

S, D, E, F = 4096, 1024, 8, 4096
C = 640                # capacity: floor(1.25*4096/8) = 640 (even)
P = 128
SC = 512               # tokens per gate chunk
N_CH = S // SC         # 8 gate chunks
S_TILES = S // P       # 32
HT = S_TILES // 2      # 16 t-tiles per routing half
D_TILES = D // P       # 8
C_TILES = C // P       # 5
FQ = F // 4            # 1024: W prefetched as 4 quarters
SENT = 100000.0

_BUILT = {}


def _build():
    nc = bacc.Bacc("TRN2", target_bir_lowering=False, debug=False, num_devices=E)

    x = nc.dram_tensor("x", [S, D], F32, kind="ExternalInput")
    xthl = nc.dram_tensor("xthl", [N_CH * P, D_TILES * 2 * SC], BF16, kind="ExternalInput")
    ghl = nc.dram_tensor("ghl", [D, 16], BF16, kind="ExternalInput")
    w = nc.dram_tensor("w", [D, F], F32R, kind="ExternalInput")
    bias_bc = nc.dram_tensor("bias_bc", [P, F], F32, kind="ExternalInput")
    selbig = nc.dram_tensor("selbig", [P, S_TILES * E], F32, kind="ExternalInput")
    ident_in = nc.dram_tensor("ident", [P, P], F32, kind="ExternalInput")
    ut128_in = nc.dram_tensor("ut128", [P, P], F32, kind="ExternalInput")
    mcarry_in = nc.dram_tensor("mcarry", [2 * S_TILES, 2 * S_TILES], F32, kind="ExternalInput")
    ones64_in = nc.dram_tensor("ones64", [2 * S_TILES, P], F32, kind="ExternalInput")
    onescol_in = nc.dram_tensor("onescol", [P, 1], F32, kind="ExternalInput")
    lo5_in = nc.dram_tensor("lo5", [P, S_TILES * 5], F32, kind="ExternalInput")
    hi5_in = nc.dram_tensor("hi5", [P, S_TILES * 5], F32, kind="ExternalInput")
    ktab_in = nc.dram_tensor("ktab", [P, S_TILES * 5], F32, kind="ExternalInput")
    iota128_in = nc.dram_tensor("iota128", [P, P], BF16, kind="ExternalInput")
    tokt_in = nc.dram_tensor("tokt", [P, S_TILES], BF16, kind="ExternalInput")
    tokp_in = nc.dram_tensor("tokp", [P, S_TILES], BF16, kind="ExternalInput")
    out = nc.dram_tensor("out", [C, F], F32, kind="ExternalOutput")

    with tile.TileContext(nc) as tc:
        with (
            tc.tile_pool(name="const", bufs=1) as cpool,
            tc.tile_pool(name="persist", bufs=1) as ppool,
            tc.tile_pool(name="rt", bufs=1) as rt_pool,
        ):
            def cload(name, src, shape, dt):
                t = cpool.tile(shape, dt, name=name)
                nc.sync.dma_start(t[:], src)
                return t

            # only what the gate stream + early mask chain needs lands first
            ident_sb = cload("ident_sb", ident_in[:, :], [P, P], F32)
            ghl_sb = cpool.tile([P, D_TILES, 16], BF16, name="ghl_sb")
            nc.sync.dma_start(ghl_sb[:], ghl[:, :].rearrange("(a p) b -> p a b", p=P))
            selbig_sb = cload("selbig_sb", selbig[:, :], [P, S_TILES * E], F32)
            actwarm = cpool.tile([1, 1], F32, name="actwarm")
            nc.scalar.copy(actwarm[:], ident_sb[0:1, 0:1])

            # persistent: logits in two half-tiles so the mask chain can start
            # on tokens 0:2048 while chunks 4-7 still stream
            logits_h = [ppool.tile([P, HT * E], F32, name=f"logits_h{h}")
                        for h in range(2)]
            dispT = ppool.tile([P, D_TILES, C], F32R, name="dispT")
            w_sb = [ppool.tile([P, D_TILES, FQ], F32R, name=f"w_sb{q}")
                    for q in range(4)]
            chosen12 = rt_pool.tile([P, 2 * S_TILES], F32, name="chosen12")

            late = {}

            def load_late_consts():
                late["ut_sb"] = cload("ut_sb", ut128_in[:, :], [P, P], F32)
                late["mcarry_sb"] = cload("mcarry_sb", mcarry_in[:, :],
                                          [2 * S_TILES, 2 * S_TILES], F32)
                late["ones64_sb"] = cload("ones64_sb", ones64_in[:, :],
                                          [2 * S_TILES, P], F32)
                late["onescol_sb"] = cload("onescol_sb", onescol_in[:, :], [P, 1], F32)
                late["lo5_sb"] = cload("lo5_sb", lo5_in[:, :], [P, S_TILES * 5], F32)
                late["hi5_sb"] = cload("hi5_sb", hi5_in[:, :], [P, S_TILES * 5], F32)
                late["ktab_sb"] = cload("ktab_sb", ktab_in[:, :], [P, S_TILES * 5], F32)
                late["iota128_sb"] = cload("iota128_sb", iota128_in[:, :], [P, P], BF16)
                late["tokt_sb"] = cload("tokt_sb", tokt_in[:, :], [P, S_TILES], BF16)
                late["tokp_sb"] = cload("tokp_sb", tokp_in[:, :], [P, S_TILES], BF16)

            load_late_consts()

            def half_chain(h):
                # top-2 masks + per-expert chosen counts for t-tiles
                # [h*16, (h+1)*16) -- emitted early so it overlaps the gate DMA
                lg = logits_h[h]
                lg3 = lg[:].rearrange("p (t e) -> p t e", e=E)
                m1 = rt_pool.tile([P, HT], F32, name=f"m1_{h}", tag="m1")
                nc.vector.reduce_max(m1[:], lg3, axis=mybir.AxisListType.X)
                mask1 = rt_pool.tile([P, HT * E], F32, name=f"mask1_{h}", tag="mask1")
                nc.vector.tensor_tensor(
                    out=mask1[:].rearrange("p (t e) -> p t e", e=E),
                    in0=lg3,
                    in1=m1[:, :, None].to_broadcast([P, HT, E]),
                    op=mybir.AluOpType.is_equal)
                negbig = rt_pool.tile([P, HT * E], F32, name=f"negbig_{h}", tag="negbig")
                nc.vector.tensor_scalar_mul(negbig[:], mask1[:], -1e9)
                masked = rt_pool.tile([P, HT * E], F32, name=f"masked_{h}", tag="masked")
                nc.vector.tensor_add(masked[:], lg[:], negbig[:])
                m2 = rt_pool.tile([P, HT], F32, name=f"m2_{h}", tag="m2")
                nc.vector.reduce_max(
                    m2[:], masked[:].rearrange("p (t e) -> p t e", e=E),
                    axis=mybir.AxisListType.X)
                mask2 = rt_pool.tile([P, HT * E], F32, name=f"mask2_{h}", tag="mask2")
                nc.vector.tensor_tensor(
                    out=mask2[:].rearrange("p (t e) -> p t e", e=E),
                    in0=masked[:].rearrange("p (t e) -> p t e", e=E),
                    in1=m2[:, :, None].to_broadcast([P, HT, E]),
                    op=mybir.AluOpType.is_equal)
                sel = selbig_sb[:, h * HT * E:(h + 1) * HT * E]
                cm1 = rt_pool.tile([P, HT * E], F32, name=f"cm1_{h}", tag="cm1")
                nc.vector.tensor_mul(cm1[:], mask1[:], sel)
                nc.vector.reduce_sum(
                    chosen12[:, h * HT:(h + 1) * HT],
                    cm1[:].rearrange("p (t e) -> p t e", e=E),
                    axis=mybir.AxisListType.X)
                cm2 = rt_pool.tile([P, HT * E], F32, name=f"cm2_{h}", tag="cm2")
                nc.vector.tensor_mul(cm2[:], mask2[:], sel)
                nc.vector.reduce_sum(
                    chosen12[:, S_TILES + h * HT:S_TILES + (h + 1) * HT],
                    cm2[:].rearrange("p (t e) -> p t e", e=E),
                    axis=mybir.AxisListType.X)

            # ---- Phase 1: gate, replicated, expert-major, chunk-pipelined ----
            with (
                tc.tile_pool(name="xc", bufs=2) as xc_pool,
                tc.tile_pool(name="gps", bufs=4, space="PSUM") as gps_pool,
                tc.tile_pool(name="lgt", bufs=4) as lgt_pool,
                tc.tile_pool(name="pt", bufs=3, space="PSUM") as pt_pool,
                tc.tile_pool(name="gwm", bufs=1, space="PSUM") as gwm_pool,
            ):
                warm_ps = gwm_pool.tile([P, E], F32, name="warm_ps")
                prev_xc = None
                for c in range(N_CH):
                    xc = xc_pool.tile([P, D_TILES, 2 * SC], BF16, name=f"xc{c}", tag="xc")
                    if c == 1:
                        # serialize chunk1's DMA behind chunk0 so chunk0 gets
                        # full SDMA bandwidth and the gate starts ~5us earlier
                        nc.vector.tensor_copy(xc[0:1, 0, 0:1], prev_xc[0:1, 0, 0:1])
                    nc.sync.dma_start(
                        xc[:],
                        xthl[c * P:(c + 1) * P, :]
                        .rearrange("p (a b) -> p a b", a=D_TILES))
                    gps = gps_pool.tile([16, SC], F32, name=f"gps{c}", tag="gps")
                    nmm = 0
                    for dt in range(D_TILES):
                        for half in range(2):
                            nc.tensor.matmul(
                                gps[:], ghl_sb[:, dt, :],
                                xc[:, dt, half * SC:(half + 1) * SC],
                                start=(nmm == 0), stop=(nmm == 15),
                                skip_group_check=True)
                            nmm += 1
                    lgt = lgt_pool.tile([16, SC], F32, name=f"lgt{c}", tag="lgt")
                    nc.vector.tensor_copy(lgt[:], gps[:])
                    for j in range(SC // P):
                        pt = pt_pool.tile([P, 16], F32, name=f"pt{c}_{j}", tag="pt")
                        nc.tensor.transpose(
                            pt[:], lgt[:, j * P:(j + 1) * P], ident_sb[:16, :16])
                        t = c * (SC // P) + j
                        # logits[p, e] = pt[:, 0:8] + pt[:, 8:16] (strided reduce)
                        nc.vector.reduce_sum(
                            logits_h[t // HT][:, (t % HT) * E:(t % HT + 1) * E],
                            pt[:].rearrange("p (a b) -> p b a", a=2),
                            axis=mybir.AxisListType.X)
                    # keep-warm: one junk matmul chained on this chunk's last
                    # logits reduce fires mid-DMA-gap, keeping the PE's HAM
                    # activity window busy (else ~7us of cold 427ns matmuls)
                    tl = c * (SC // P) + 3
                    nc.tensor.matmul(
                        warm_ps[:], ident_sb[:],
                        logits_h[tl // HT][:, (tl % HT) * E:(tl % HT + 1) * E],
                        start=True, stop=True, skip_group_check=True)
                    prev_xc = xc
                    if c == N_CH // 2 - 1:
                        half_chain(0)
                    if c == N_CH - 2:
                        nc.vector.tensor_copy(w_sb[0][0:1, 0, 0:1],
                                              xc[0:1, 0, 0:1])
                        nc.sync.dma_start(
                            w_sb[0][:],
                            w[:, 0:FQ].rearrange("(a p) b -> p a b", p=P))
                    if c == N_CH - 1:
                        # release the rest of W behind the last gate chunk:
                        # 1-element copies create the ordering dep so the W
                        # flood doesn't steal SDMA bandwidth from the gate
                        for q in range(1, 4):
                            nc.vector.tensor_copy(w_sb[q][0:1, 0, 0:1],
                                                  xc[0:1, 0, 0:1])
                        for q in range(1, 4):
                            nc.sync.dma_start(
                                w_sb[q][:],
                                w[:, q * FQ:(q + 1) * FQ]
                                .rearrange("(a p) b -> p a b", p=P))

                half_chain(1)

            # ---- Phase 2: routing tail ----
            prt_cm = tc.tile_pool(name="prt", bufs=1, space="PSUM")
            prt_pool = prt_cm.__enter__()
            inv_cm = tc.tile_pool(name="inv", bufs=1)
            inv_pool = inv_cm.__enter__()

            ut_sb = late["ut_sb"]; mcarry_sb = late["mcarry_sb"]
            ones64_sb = late["ones64_sb"]; onescol_sb = late["onescol_sb"]
            lo5_sb = late["lo5_sb"]; hi5_sb = late["hi5_sb"]
            ktab_sb = late["ktab_sb"]; iota128_sb = late["iota128_sb"]
            tokt_sb = late["tokt_sb"]; tokp_sb = late["tokp_sb"]

            # cumsum over tokens: intra-tile (UT128) + cross-tile carries
            ps_r = prt_pool.tile([P, 2 * S_TILES], F32, name="ps_r")
            nc.tensor.matmul(ps_r[:], ut_sb[:], chosen12[:],
                             start=True, stop=False, skip_group_check=True)
            ps_tot = prt_pool.tile([2 * S_TILES, 1], F32, name="ps_tot")
            nc.tensor.matmul(ps_tot[:], chosen12[:], onescol_sb[:],
                             start=True, stop=True)
            totcol = rt_pool.tile([2 * S_TILES, 1], F32, name="totcol")
            nc.vector.tensor_copy(totcol[:], ps_tot[:])
            rmat = rt_pool.tile([2 * S_TILES, 2 * S_TILES], F32, name="rmat")
            nc.vector.tensor_mul(
                rmat[:], totcol[:].to_broadcast([2 * S_TILES, 2 * S_TILES]),
                mcarry_sb[:])
            nc.tensor.matmul(ps_r[:], ones64_sb[:], rmat[:],
                             start=False, stop=True, skip_group_check=True)

            # slot = ch1*cum1 + ch2*cum2 - 1 + (1-ch1-ch2)*SENT
            u1 = rt_pool.tile([P, S_TILES], F32, name="u1")
            nc.vector.tensor_mul(u1[:], chosen12[:, 0:S_TILES], ps_r[:, 0:S_TILES])
            u2 = rt_pool.tile([P, S_TILES], F32, name="u2")
            nc.vector.tensor_mul(u2[:], chosen12[:, S_TILES:2 * S_TILES],
                                 ps_r[:, S_TILES:2 * S_TILES])
            u12 = rt_pool.tile([P, S_TILES], F32, name="u12")
            nc.vector.tensor_add(u12[:], u1[:], u2[:])
            vv = rt_pool.tile([P, S_TILES], F32, name="vv")
            nc.vector.tensor_add(vv[:], chosen12[:, 0:S_TILES],
                                 chosen12[:, S_TILES:2 * S_TILES])
            vs = rt_pool.tile([P, S_TILES], F32, name="vs")
            nc.vector.tensor_scalar_mul(vs[:], vv[:], SENT)
            wd = rt_pool.tile([P, S_TILES], F32, name="wd")
            nc.vector.tensor_sub(wd[:], u12[:], vs[:])
            slots_f = rt_pool.tile([P, S_TILES], F32, name="slots_f")
            nc.vector.tensor_scalar_add(slots_f[:], wd[:], SENT - 1.0)

            # ---- two-level inverse map: slot -> token ----
            sl3 = slots_f[:, :, None].to_broadcast([P, S_TILES, 5])
            lo3 = lo5_sb[:].rearrange("p (t k) -> p t k", k=5)
            hi3 = hi5_sb[:].rearrange("p (t k) -> p t k", k=5)
            ge = inv_pool.tile([P, S_TILES * 5], F32, name="ge")
            nc.vector.tensor_tensor(
                out=ge[:].rearrange("p (t k) -> p t k", k=5), in0=sl3, in1=lo3,
                op=mybir.AluOpType.is_ge)
            le = inv_pool.tile([P, S_TILES * 5], F32, name="le")
            nc.vector.tensor_tensor(
                out=le[:].rearrange("p (t k) -> p t k", k=5), in0=sl3, in1=hi3,
                op=mybir.AluOpType.is_le)
            ohct = inv_pool.tile([P, S_TILES * 5], F32, name="ohct")
            nc.vector.tensor_mul(ohct[:], ge[:], le[:])
            ctk = inv_pool.tile([P, S_TILES * 5], F32, name="ctk")
            nc.vector.tensor_mul(ctk[:], ohct[:], ktab_sb[:])
            ctv = rt_pool.tile([P, S_TILES], F32, name="ctv")
            nc.vector.reduce_sum(
                ctv[:], ctk[:].rearrange("p (t k) -> p t k", k=5),
                axis=mybir.AxisListType.X)
            ct128 = rt_pool.tile([P, S_TILES], F32, name="ct128")
            nc.vector.tensor_scalar_mul(ct128[:], ctv[:], 128.0)
            sr = rt_pool.tile([P, S_TILES], F32, name="sr")
            nc.vector.tensor_sub(sr[:], slots_f[:], ct128[:])
            srb = rt_pool.tile([P, S_TILES], BF16, name="srb")
            nc.vector.tensor_copy(srb[:], sr[:])
            oh_r = inv_pool.tile([P, S_TILES * P], BF16, name="oh_r")
            nc.vector.tensor_tensor(
                out=oh_r[:].rearrange("p (t r) -> p t r", r=P),
                in0=iota128_sb[:, None, :].to_broadcast([P, S_TILES, P]),
                in1=srb[:, :, None].to_broadcast([P, S_TILES, P]),
                op=mybir.AluOpType.is_equal)
            ohct_b = inv_pool.tile([P, S_TILES * 5], BF16, name="ohct_b")
            nc.vector.tensor_copy(ohct_b[:], ohct[:])
            AB = inv_pool.tile([P, S_TILES, 15], BF16, name="AB")
            nc.vector.tensor_tensor(
                out=AB[:, :, 0:5],
                in0=ohct_b[:].rearrange("p (t k) -> p t k", k=5),
                in1=tokt_sb[:, :, None].to_broadcast([P, S_TILES, 5]),
                op=mybir.AluOpType.mult)
            nc.vector.tensor_tensor(
                out=AB[:, :, 5:10],
                in0=ohct_b[:].rearrange("p (t k) -> p t k", k=5),
                in1=tokp_sb[:, :, None].to_broadcast([P, S_TILES, 5]),
                op=mybir.AluOpType.mult)
            nc.vector.tensor_copy(
                AB[:, :, 10:15], ohct_b[:].rearrange("p (t k) -> p t k", k=5))
            ps_inv = prt_pool.tile([P, 15], F32, name="ps_inv")
            for t in range(S_TILES):
                nc.tensor.matmul(
                    ps_inv[:], oh_r[:, t * P:(t + 1) * P], AB[:, t, :],
                    start=(t == 0), stop=(t == S_TILES - 1),
                    skip_group_check=True)
            tokf = rt_pool.tile([P, 5], F32, name="tokf")
            nc.vector.tensor_scalar_mul(tokf[:], ps_inv[:, 0:5], 128.0)
            tokf2 = rt_pool.tile([P, 5], F32, name="tokf2")
            nc.vector.tensor_add(tokf2[:], tokf[:], ps_inv[:, 5:10])
            invv = rt_pool.tile([P, 5], F32, name="invv")
            nc.vector.tensor_scalar(
                out=invv[:], in0=ps_inv[:, 10:15], scalar1=-1e6, scalar2=1e6,
                op0=mybir.AluOpType.mult, op1=mybir.AluOpType.add)
            slf = rt_pool.tile([P, 5], F32, name="slf")
            nc.vector.tensor_add(slf[:], tokf2[:], invv[:])
            sl_i = rt_pool.tile([P, 5], I32, name="sl_i")
            nc.vector.tensor_copy(sl_i[:], slf[:])

            inv_cm.__exit__(None, None, None)
            prt_cm.__exit__(None, None, None)

            # ---- gather + transpose + FFN, pipelined per c-tile ----
            with (
                tc.tile_pool(name="po", bufs=6, space="PSUM") as po_pool,
                tc.tile_pool(name="osb", bufs=6) as osb_pool,
                tc.tile_pool(name="bia", bufs=1) as bia_pool,
            ):
                bias_sb = bia_pool.tile([P, F], F32, name="bias_sb")
                nc.sync.dma_start(bias_sb[:], bias_bc[:, :])

                def ffn_block(g, ct):
                    pss = [po_pool.tile([P, 512], F32, name=f"po{g}_{ct}_{f4}",
                                        tag="po")
                           for f4 in range(4)]
                    for dt in range(D_TILES):
                        lhs = dispT[:, dt, ct * P:(ct + 1) * P]
                        for f4 in range(4):
                            q = 2 * g + f4 // 2
                            fo = (f4 % 2) * 512
                            nc.tensor.matmul(
                                pss[f4][:], lhs,
                                w_sb[q][:, dt, fo:fo + 512],
                                start=(dt == 0), stop=(dt == D_TILES - 1),
                                skip_group_check=True)
                    for f4 in range(4):
                        fcol = (g * 4 + f4) * 512
                        o_sb = osb_pool.tile([P, 512], F32,
                                             name=f"o{g}_{ct}_{f4}", tag="osb")
                        nc.vector.tensor_add(o_sb[:], pss[f4][:],
                                             bias_sb[:, fcol:fcol + 512])
                        nc.scalar.dma_start(
                            out[ct * P:(ct + 1) * P, fcol:fcol + 512], o_sb[:])

                with (
                    tc.tile_pool(name="disp", bufs=3) as disp_pool,
                    tc.tile_pool(name="ptr", bufs=2, space="PSUM") as ptr_pool,
                ):
                    for ct in range(C_TILES):
                        disp_sb = disp_pool.tile([P, D], F32, name=f"disp{ct}",
                                                 tag="disp")
                        nc.gpsimd.memset(disp_sb[:], 0)
                        nc.gpsimd.indirect_dma_start(
                            out=disp_sb[:],
                            out_offset=None,
                            in_=x[:, :],
                            in_offset=bass.IndirectOffsetOnAxis(
                                ap=sl_i[:, ct:ct + 1], axis=0),
                            bounds_check=S - 1,
                            oob_is_err=False)
                        for dt in range(D_TILES):
                            pst = ptr_pool.tile([P, P], F32,
                                                name=f"pst{ct}_{dt}", tag="pst")
                            nc.tensor.transpose(
                                pst[:], disp_sb[:, dt * P:(dt + 1) * P],
                                ident_sb[:])
                            # all dispT copies on ACT: DVE's FIFO otherwise
                            # serializes them behind the previous block's FFN
                            # bias-adds, idling the PE ~1.7us per c-tile
                            nc.scalar.copy(
                                dispT[:, dt, ct * P:(ct + 1) * P], pst[:])
                        ffn_block(0, ct)
                for ct in range(C_TILES):
                    ffn_block(1, ct)

    nc.compile()
    return nc


def _consts():
    ident = np.eye(P, dtype=np.float32)
    ut128 = np.triu(np.ones((P, P), dtype=np.float32))
    n = S_TILES
    slt = np.triu(np.ones((n, n), dtype=np.float32), k=1)
    mcarry = np.zeros((2 * n, 2 * n), dtype=np.float32)
    mcarry[:n, :n] = slt
    mcarry[:n, n:] = 1.0
    mcarry[n:, n:] = slt
    ones64 = np.ones((2 * n, P), dtype=np.float32)
    onescol = np.ones((P, 1), dtype=np.float32)
    lo5 = np.broadcast_to((np.arange(5, dtype=np.float32) * 128)[None, None, :],
                          (P, n, 5)).reshape(P, n * 5).copy()
    hi5 = lo5 + 127.0
    ktab = np.broadcast_to(np.arange(5, dtype=np.float32)[None, None, :],
                           (P, n, 5)).reshape(P, n * 5).copy()
    iota128 = np.broadcast_to(np.arange(P, dtype=np.float32)[None, :],
                              (P, P)).astype(ml_dtypes.bfloat16)
    tokt = np.broadcast_to(np.arange(n, dtype=np.float32)[None, :],
                           (P, n)).astype(ml_dtypes.bfloat16)
    tokp = np.broadcast_to(np.arange(P, dtype=np.float32)[:, None],
                           (P, n)).astype(ml_dtypes.bfloat16)
    return dict(ident=ident, ut128=ut128, mcarry=mcarry, ones64=ones64,
                onescol=onescol, lo5=lo5, hi5=hi5, ktab=ktab,
                iota128=iota128, tokt=tokt, tokp=tokp)


def kernel(x, gate_w, weight, bias, _trace=False):
    if "nc" not in _BUILT:
        _BUILT["nc"] = _build()
    nc = _BUILT["nc"]

    bf16 = ml_dtypes.bfloat16
    x = np.ascontiguousarray(x, dtype=np.float32)
    xt = np.ascontiguousarray(x.T)                          # [D, S]
    xh = xt.astype(bf16)
    xl = (xt - xh.astype(np.float32)).astype(bf16)
    # chunk-contiguous layout: [c*P + p, (dt, half, s)] so each gate chunk is
    # one contiguous 2.1MB DMA with 16KB descriptors
    xh5 = xh.reshape(D_TILES, P, N_CH, SC).transpose(2, 1, 0, 3)  # [c, p, dt, s]
    xl5 = xl.reshape(D_TILES, P, N_CH, SC).transpose(2, 1, 0, 3)
    xthl = np.ascontiguousarray(
        np.stack([xh5, xl5], axis=3)                        # [c, p, dt, 2, s]
        .reshape(N_CH * P, D_TILES * 2 * SC))
    gwt = gate_w.T.astype(np.float32)                       # [D, E]
    gh = gwt.astype(bf16)
    gl = (gwt - gh.astype(np.float32)).astype(bf16)
    ghl = np.ascontiguousarray(np.concatenate([gh, gl], axis=1))  # [D, 16]
    consts = _consts()

    bias_f = bias.reshape(E, F).astype(np.float32)

    in_maps = []
    for e in range(E):
        sel = np.zeros((P, S_TILES * E), dtype=np.float32)
        sel[:, e::E] = 1.0
        m = dict(x=x, xthl=xthl, ghl=ghl,
                 w=np.ascontiguousarray(weight[e].astype(np.float32)),
                 bias_bc=np.ascontiguousarray(
                     np.broadcast_to(bias_f[e][None, :], (P, F))),
                 selbig=sel, **consts)
        in_maps.append(m)

    kw = {}
    if _trace:
        import types, sys
        from trn_agent_boot.trn_boot import _ntff_profile_via_ctypes
        hook = _ntff_profile_via_ctypes('/opt/axon/libaxon_pjrt.so')
        mod = types.ModuleType('antenv.axon_hooks')
        mod.get_axon_ntff_profile_hook = lambda: hook
        sys.modules['antenv.axon_hooks'] = mod
        kw["trace"] = True

    res = run_bass_kernel_spmd(nc, in_maps, core_ids=list(range(E)), **kw)
    _BUILT["last_res"] = res
    out = np.stack([res.results[e]["out"] for e in range(E)]).astype(np.float32)
    if _trace:
        return out, res
    return out
